# revision 19
# baseline (speedup 1.0000x reference)
"""Trainium2 Bass kernel for nn_ConditionalMomentEncoder.

Self-contained: takes full unsharded inputs, shards batch-parallel over 8
NeuronCores, runs one Bass/Tile program per core, gathers [32, 512] output.

Math notes (vs the jax reference):
- The ragged argsort/gather is eliminated: invalid slots have fmp == 0, so
  their key rows and logits are exactly 0. Attention over all N=128 slots
  with denominator correction  den = rowsum(exp(logits)) - (N - L)
  reproduces the reference's softmax over its L=96 gathered slots exactly.
- Softmax max-subtraction is skipped: logits are bounded, fp32 exp is safe.
- conv1 fold: conv1 is linear in its input channels and its input is
  O = A @ K (A = scaled attention weights [224, 128], K = raw keys
  [128, 4096] viewed as 128 images of 64x64). So per batch we compute
  W'[n, :] = sum_m A[m, n] * w1r[m, :]  (a [224]->[128] channel fold of the
  conv weights) and run conv1 directly on the keys images with W'. This
  skips the O matmuls entirely and roughly halves conv1's contraction.
- A is materialized via a PE transpose (matmul with identity rhs) of
  E^T[n, m] * s[n] (s = fmp/eig), scaled by 1/den[m] on evacuation.

Scheduling notes:
- PE executes in program order; phases are software-pipelined so that
  batch b+1's independent matmuls cover batch b's activation/evac stalls.
- A short warm-up matmul stream runs during the initial input DMA so the
  PE HAM clock gate is already at 2.4 GHz when real work arrives.
- The flatten transpose for the final linear is one small SBUF->SBUF DMA
  per batch (contiguous 16B runs per partition); the feature permutation
  is absorbed into the host-side lin_w packing.
"""

import sys

sys.path.insert(0, "/opt/trn_rl_repo")

import ml_dtypes
import numpy as np

import concourse.bacc as bacc
import concourse.mybir as mybir
import concourse.tile as tile
from concourse import bass_utils

F32 = mybir.dt.float32
F32R = mybir.dt.float32r
BF16 = mybir.dt.bfloat16
AF = mybir.ActivationFunctionType
ALU = mybir.AluOpType

B, N, D = 32, 128, 4096
L = 96
IMG = 64
NCORES = 8
BL = B // NCORES  # 4 batches per core
SCALE = float(D) ** -0.5
M_ALL = L + N  # 224 query rows total
M_PAD = 224

_CACHE = {}
LAST_RESULT = None


def _build_program():
    nc = bacc.Bacc("TRN2", target_bir_lowering=False, debug=False)

    keys_d = nc.dram_tensor("keys", [BL, N, D], BF16, kind="ExternalInput")
    keysT_d = nc.dram_tensor("keysT", [BL, N, D], BF16, kind="ExternalInput")
    scales_d = nc.dram_tensor("scales", [128, BL * 4], F32, kind="ExternalInput")
    qt_d = nc.dram_tensor("qt", [128, 32 * M_PAD], BF16, kind="ExternalInput")
    w1f_d = nc.dram_tensor("w1f", [96, 9 * 256], BF16, kind="ExternalInput")
    w1e_d = nc.dram_tensor("w1e", [128, 9 * 256], BF16, kind="ExternalInput")
    w2a_d = nc.dram_tensor("w2a", [128, 9 * 128], BF16, kind="ExternalInput")
    w2b_d = nc.dram_tensor("w2b", [128, 9 * 128], BF16, kind="ExternalInput")
    w3_d = nc.dram_tensor("w3", [128, 9 * 128], BF16, kind="ExternalInput")
    w4_d = nc.dram_tensor("w4", [128, 9 * 64], BF16, kind="ExternalInput")
    lw_d = nc.dram_tensor("lw", [128, 8 * 512], BF16, kind="ExternalInput")
    bias_d = nc.dram_tensor("biasp", [128, 5], F32, kind="ExternalInput")
    linb_d = nc.dram_tensor("linb", [2, 512], BF16, kind="ExternalInput")
    onesc_d = nc.dram_tensor("onesc", [128, 2], F32R, kind="ExternalInput")
    onesr_d = nc.dram_tensor("onesr", [2, 4], BF16, kind="ExternalInput")
    ident_d = nc.dram_tensor("ident", [128, 128], BF16, kind="ExternalInput")
    out_d = nc.dram_tensor("out", [BL, 512], F32, kind="ExternalOutput")
    warm_d = nc.dram_tensor("warm", [1, 1], F32, kind="ExternalOutput")

    with tile.TileContext(nc) as tc:
        with (
            tc.tile_pool(name="const", bufs=1) as cpool,
            tc.tile_pool(name="imgs", bufs=2) as ipool,
            tc.tile_pool(name="flat", bufs=1) as fpool,
            tc.tile_pool(name="keys", bufs=4) as kpool,
            tc.tile_pool(name="keysT", bufs=3) as ktpool,
            tc.tile_pool(name="small", bufs=4) as spool,
            tc.tile_pool(name="wprime", bufs=4) as wpool,
            tc.tile_pool(name="gt_ps", bufs=2, space="PSUM") as gt_psum,
            tc.tile_pool(name="tr_ps", bufs=1, space="PSUM") as tr_psum,
            tc.tile_pool(name="wp_ps", bufs=2, space="PSUM") as wp_psum,
            tc.tile_pool(name="cv_ps", bufs=3, space="PSUM") as cv_psum,
        ):
            # ---- startup DMAs: dispatch cost is ~0.6us per dma_start,
            # serialized per issuing engine, so keep the count low and split
            # across the two HWDGE queues (sync + scalar). The sync queue
            # carries the latency-critical front stream. ----
            ident = cpool.tile([128, 128], BF16)
            nc.sync.dma_start(ident[:], ident_d[:])
            keysT0 = ktpool.tile([128, D], BF16, tag="kt")
            qt_sb = cpool.tile([128, 32 * M_PAD], BF16)
            nc.sync.dma_start(keysT0[:, 0:2048], keysT_d[0][:, 0:2048])
            nc.sync.dma_start(qt_sb[:, 0:3584], qt_d[:, 0:3584])
            nc.sync.dma_start(keysT0[:, 2048:4096], keysT_d[0][:, 2048:4096])
            nc.sync.dma_start(qt_sb[:, 3584:7168], qt_d[:, 3584:7168])
            ktt = {0: keysT0}
            kst = {}
            t = ktpool.tile([128, D], BF16, tag="kt")
            nc.sync.dma_start(t[:], keysT_d[1])
            ktt[1] = t
            k = kpool.tile([128, D], BF16, tag="k")
            nc.sync.dma_start(k[:], keys_d[0])
            kst[0] = k
            t = ktpool.tile([128, D], BF16, tag="kt")
            nc.sync.dma_start(t[:], keysT_d[2])
            ktt[2] = t
            t = ktpool.tile([128, D], BF16, tag="kt")
            nc.sync.dma_start(t[:], keysT_d[3])
            ktt[3] = t
            for pb in (1, 2, 3):
                k = kpool.tile([128, D], BF16, tag="k")
                nc.sync.dma_start(k[:], keys_d[pb])
                kst[pb] = k

            # scalar queue: everything else, ordered by first consumption
            scp = cpool.tile([128, BL * 4], F32)
            nc.scalar.dma_start(scp[:], scales_d[:])
            onesc = cpool.tile([128, 2], F32R)
            nc.scalar.dma_start(onesc[:], onesc_d[:])
            w1f = cpool.tile([96, 9 * 256], BF16)
            nc.scalar.dma_start(w1f[:], w1f_d[:])
            w1e = cpool.tile([128, 9 * 256], BF16)
            nc.scalar.dma_start(w1e[:], w1e_d[:])
            biasp = cpool.tile([128, 5], F32)
            nc.scalar.dma_start(biasp[:], bias_d[:])
            w2a = cpool.tile([128, 9 * 128], BF16)
            nc.scalar.dma_start(w2a[:], w2a_d[:])
            w2b = cpool.tile([128, 9 * 128], BF16)
            nc.scalar.dma_start(w2b[:], w2b_d[:])
            w3 = cpool.tile([128, 9 * 128], BF16)
            nc.scalar.dma_start(w3[:], w3_d[:])
            w4 = cpool.tile([128, 9 * 64], BF16)
            nc.scalar.dma_start(w4[:], w4_d[:])
            lw = cpool.tile([128, 8 * 512], BF16)
            nc.scalar.dma_start(lw[:], lw_d[:])
            linb = cpool.tile([2, 512], BF16)
            nc.scalar.dma_start(linb[:], linb_d[:])
            ones_row = cpool.tile([2, 4], BF16)
            nc.scalar.dma_start(ones_row[:], onesr_d[:])

            # flat features + transposed copy for the linear
            f_all = fpool.tile([64, BL * 16], BF16)
            fr = fpool.tile([128, 4 * 8], BF16)  # col = 8*b + j

            def rtap(ky, kx, S):
                # restricted tap for unpadded stride-2 conv, output side S
                oy0 = 1 if ky == 0 else 0
                ox0 = 1 if kx == 0 else 0
                return (oy0, S - oy0, ox0, S - ox0,
                        2 * oy0 + ky - 1, 2 * ox0 + kx - 1)
            TAPS1 = [(1, 1), (0, 0), (0, 1), (0, 2), (1, 0), (1, 2),
                     (2, 0), (2, 1), (2, 2)]

            # W' chunk boundaries (psum bank = 512 fp32)
            WCH = [(0, 512), (512, 1024), (1024, 1536), (1536, 2048),
                   (2048, 2304)]

            # ---- PE warm-up: ~38 matmuls on the identity while input DMAs
            # stream in, so HAM is at 2.4 GHz when real work starts ----
            warm_ps = cv_psum.tile([128, 128], F32, tag="cv")
            NWARM = 24
            for i in range(NWARM):
                nc.tensor.matmul(warm_ps[:], ident[:], ident[:],
                                 start=(i == 0), stop=(i == NWARM - 1))
            warm_sb = cpool.tile([1, 1], F32)
            nc.vector.tensor_copy(warm_sb[:], warm_ps[0:1, 0:1])
            nc.sync.dma_start(warm_d[:], warm_sb[:])

            # ======== phase A: attention + weight fold (pipelined) ========
            def logits(b):
                gt_ps = gt_psum.tile([128, M_PAD], F32, tag="gt")
                keysT_sb = ktt[b]
                for c in range(32):
                    nc.tensor.matmul(
                        gt_ps[:],
                        keysT_sb[:, c * 128:(c + 1) * 128],
                        qt_sb[:, c * M_PAD:(c + 1) * M_PAD],
                        start=(c == 0),
                        stop=(c == 31),
                    )
                return gt_ps

            def dentr(b, gt_ps):
                # den + PE transpose of scaled E^T -> A[m, n] (bf16)
                # E^T = exp(G^T * scale_col)
                et = spool.tile([128, M_ALL], F32R, tag="et")
                nc.scalar.activation(et[:, 0:96], gt_ps[:, 0:96], AF.Exp,
                                     bias=0.0, scale=scp[:, 4 * b:4 * b + 1])
                nc.scalar.activation(et[:, 96:224], gt_ps[:, 96:224], AF.Exp,
                                     bias=0.0, scale=scp[:, 4 * b + 1:4 * b + 2])

                # den[m] = sum_n E^T[n, m] (ones matmul)
                den_f_ps = tr_psum.tile([96, 2], F32, tag="tr")
                nc.tensor.matmul(den_f_ps[:], et[:, 0:96], onesc[:],
                                 start=True, stop=True)
                den_e_ps = tr_psum.tile([128, 2], F32, tag="tr")
                nc.tensor.matmul(den_e_ps[:], et[:, 96:224], onesc[:],
                                 start=True, stop=True)

                rden = spool.tile([128, 3], F32, tag="rden")
                nc.vector.tensor_scalar_add(rden[0:96, 0:1], den_f_ps[:, 0:1],
                                            -32.0)
                nc.vector.reciprocal(rden[0:96, 1:2], rden[0:96, 0:1])
                nc.vector.reciprocal(rden[0:128, 2:3], den_e_ps[:, 0:1])

                # scaled E^T (bf16): et2[n, m] = E^T[n, m] * s[n]
                et2 = spool.tile([128, M_ALL], BF16, tag="et2")
                nc.vector.tensor_scalar_mul(et2[:, 0:96], et[:, 0:96],
                                            scp[:, 4 * b + 2:4 * b + 3])
                nc.vector.tensor_scalar_mul(et2[:, 96:224], et[:, 96:224],
                                            scp[:, 4 * b + 3:4 * b + 4])

                # A[m, n] = et2^T[m, n] / den[m] via PE transpose
                tr_f_ps = tr_psum.tile([96, 128], F32, tag="tr")
                nc.tensor.matmul(tr_f_ps[:], et2[:, 0:96], ident[:],
                                 start=True, stop=True)
                tr_e_ps = tr_psum.tile([128, 128], F32, tag="tr")
                nc.tensor.matmul(tr_e_ps[:], et2[:, 96:224], ident[:],
                                 start=True, stop=True)
                at_f = spool.tile([96, 128], BF16, tag="atf")
                nc.scalar.activation(at_f[:], tr_f_ps[:], AF.Copy,
                                     bias=0.0, scale=rden[0:96, 1:2])
                at_e = spool.tile([128, 128], BF16, tag="ate")
                nc.scalar.activation(at_e[:], tr_e_ps[:], AF.Copy,
                                     bias=0.0, scale=rden[0:128, 2:3])
                return at_f, at_e

            def wfold(ats):
                # W'[n, 9*256] = A_f^T @ w1f + A_e^T @ w1e
                at_f, at_e = ats
                wp = wpool.tile([128, 9 * 256], BF16, tag="wp")
                for c0, c1 in WCH:
                    wp_ps = wp_psum.tile([128, 512], F32, tag="wp")
                    nc.tensor.matmul(wp_ps[:, 0:c1 - c0], at_f[:],
                                     w1f[:, c0:c1], start=True, stop=False)
                    nc.tensor.matmul(wp_ps[:, 0:c1 - c0], at_e[:],
                                     w1e[:, c0:c1], start=False, stop=True)
                    nc.vector.tensor_copy(wp[:, c0:c1], wp_ps[:, 0:c1 - c0])
                return wp

            gts = {}
            ats = {}
            wpt = {}

            # ======== phase C: conv stack (cross-batch interleaved) ========
            c1t = {}

            def conv1_quarter(b, og, fc):
                # conv1': 128 keys-ch -> 128 of 256 ch, 64x64 -> 32x32
                if b not in c1t:
                    ca = ipool.tile([128, 1024], BF16, tag="c1a")
                    cb = ipool.tile([128, 1024], BF16, tag="c1b")
                    c1t[b] = (ca, cb)
                wp = wpt[b]
                keys3 = kst[b][:].rearrange("p (h w) -> p h w", h=64, w=64)
                ps = cv_psum.tile([128, 512], F32, tag="cv")
                ps3 = ps[:].rearrange("p (a b) -> p a b", a=16, b=32)
                for ti, (ky, kx) in enumerate(TAPS1):
                    oy0 = 1 if (ky == 0 and fc == 0) else 16 * fc
                    ny = 16 * (fc + 1) - oy0
                    ox0 = 1 if kx == 0 else 0
                    nx = 32 - ox0
                    iy0 = 2 * oy0 + ky - 1
                    ix0 = 2 * ox0 + kx - 1
                    nc.tensor.matmul(
                        ps3[:, oy0 - 16 * fc:oy0 - 16 * fc + ny, ox0:32],
                        wp[:, (3 * ky + kx) * 256 + og * 128:
                           (3 * ky + kx) * 256 + og * 128 + 128],
                        keys3[:, iy0:iy0 + 2 * ny - 1:2,
                              ix0:ix0 + 2 * nx - 1:2],
                        start=(ti == 0),
                        stop=(ti == 8),
                    )
                c1dst = c1t[b][og]
                nc.scalar.activation(
                    c1dst[:, 512 * fc:512 * (fc + 1)], ps[:],
                    AF.Relu, bias=biasp[:, og:og + 1], scale=1.0,
                )

            def conv2(b):
                ca, cb = c1t[b]
                c1a3 = ca[:].rearrange("p (h w) -> p h w", h=32, w=32)
                c1b3 = cb[:].rearrange("p (h w) -> p h w", h=32, w=32)
                ps = cv_psum.tile([128, 256], F32, tag="cv")
                ps3 = ps[:].rearrange("p (a b) -> p a b", a=16, b=16)
                for ti, (ky, kx) in enumerate(TAPS1):
                    rr = rtap(ky, kx, 16)
                    for ic in range(2):
                        srci = c1a3 if ic == 0 else c1b3
                        wt = w2a if ic == 0 else w2b
                        nc.tensor.matmul(
                            ps3[:, rr[0]:rr[0] + rr[1], rr[2]:rr[2] + rr[3]],
                            wt[:, (3 * ky + kx) * 128:(3 * ky + kx) * 128 + 128],
                            srci[:, rr[4]:rr[4] + 2 * rr[1] - 1:2,
                                 rr[5]:rr[5] + 2 * rr[3] - 1:2],
                            start=(ti == 0 and ic == 0),
                            stop=(ti == 8 and ic == 1),
                        )
                c2o = ipool.tile([128, 256], BF16, tag="c2o")
                nc.scalar.activation(c2o[:], ps[:], AF.Relu,
                                     bias=biasp[:, 2:3], scale=1.0)
                return c2o

            def conv3(b, c2o):
                c2o3 = c2o[:].rearrange("p (h w) -> p h w", h=16, w=16)
                ps = cv_psum.tile([128, 64], F32, tag="cv")
                ps3 = ps[:].rearrange("p (a b) -> p a b", a=8, b=8)
                for ti, (ky, kx) in enumerate(TAPS1):
                    rr = rtap(ky, kx, 8)
                    nc.tensor.matmul(
                        ps3[:, rr[0]:rr[0] + rr[1], rr[2]:rr[2] + rr[3]],
                        w3[:, (3 * ky + kx) * 128:(3 * ky + kx) * 128 + 128],
                        c2o3[:, rr[4]:rr[4] + 2 * rr[1] - 1:2,
                             rr[5]:rr[5] + 2 * rr[3] - 1:2],
                        start=(ti == 0), stop=(ti == 8),
                    )
                c3o = ipool.tile([128, 64], BF16, tag="c3o")
                nc.scalar.activation(c3o[:], ps[:], AF.Relu,
                                     bias=biasp[:, 3:4], scale=1.0)
                return c3o

            def conv4(b, c3o):
                c3o3 = c3o[:].rearrange("p (h w) -> p h w", h=8, w=8)
                ps = cv_psum.tile([64, 16], F32, tag="cv")
                ps3 = ps[:].rearrange("p (a b) -> p a b", a=4, b=4)
                for ti, (ky, kx) in enumerate(TAPS1):
                    rr = rtap(ky, kx, 4)
                    nc.tensor.matmul(
                        ps3[:, rr[0]:rr[0] + rr[1], rr[2]:rr[2] + rr[3]],
                        w4[:, (3 * ky + kx) * 64:(3 * ky + kx) * 64 + 64],
                        c3o3[:, rr[4]:rr[4] + 2 * rr[1] - 1:2,
                             rr[5]:rr[5] + 2 * rr[3] - 1:2],
                        start=(ti == 0), stop=(ti == 8),
                    )
                nc.scalar.activation(f_all[:, b * 16:(b + 1) * 16], ps3[:],
                                     AF.Relu, bias=biasp[0:64, 4:5], scale=1.0)
                # feature transpose to 128 partitions, one SBUF->SBUF DMA:
                # (ch, s) -> (p = 2*ch + s//8, col = 8*b + s%8); lw host
                # packing matches this ordering.
                nc.sync.dma_start(fr[:, b * 8:(b + 1) * 8],
                                  f_all[:, b * 16:(b + 1) * 16])

            # ---- pipelined emission (PE executes in program order):
            # the logits chain is DMA-paced at the front, so batch-0's conv
            # quarters and later batches' den/fold work are interleaved to
            # keep the PE dense; conv2..4 of batch b sit between batch b+1's
            # conv1 quarters so relu-evac latencies are covered.
            gts[0] = logits(0)
            gts[1] = logits(1)
            ats[0] = dentr(0, gts[0])
            wpt[0] = wfold(ats[0])
            gts[2] = logits(2)
            ats[1] = dentr(1, gts[1])
            wpt[1] = wfold(ats[1])
            conv1_quarter(0, 0, 0)
            conv1_quarter(0, 0, 1)
            gts[3] = logits(3)
            conv1_quarter(0, 1, 0)
            ats[2] = dentr(2, gts[2])
            wpt[2] = wfold(ats[2])
            conv1_quarter(0, 1, 1)
            ats[3] = dentr(3, gts[3])
            wpt[3] = wfold(ats[3])
            for b in range(BL):
                if b + 1 < BL:
                    conv1_quarter(b + 1, 0, 0)
                    conv1_quarter(b + 1, 0, 1)
                    c2o = conv2(b)
                    conv1_quarter(b + 1, 1, 0)
                    c3o = conv3(b, c2o)
                    conv1_quarter(b + 1, 1, 1)
                    conv4(b, c3o)
                else:
                    c2o = conv2(b)
                    c3o = conv3(b, c2o)
                    conv4(b, c3o)

            # ---- linear: out[b, o] = sum_f flat[b, f] lin_w[o, f] + lin_b ----
            lin_ps = cv_psum.tile([4, 512], F32, tag="cv")
            for j in range(8):
                nc.tensor.matmul(
                    lin_ps[:],
                    fr[:, j::8],
                    lw[:, 512 * j:512 * j + 512],
                    start=(j == 0), stop=False,
                )
            nc.tensor.matmul(lin_ps[:], ones_row[:], linb[:],
                             start=False, stop=True)
            out_sb = cpool.tile([4, 512], F32)
            nc.vector.tensor_copy(out_sb[:], lin_ps[:])
            nc.sync.dma_start(out_d[:], out_sb[:])

    nc.finalize()
    return nc


def _prep_inputs(inputs):
    keys = np.ascontiguousarray(inputs["keys"], dtype=np.float32)
    fmp = np.asarray(inputs["first_moment_projections"], dtype=np.float32)
    eig = np.asarray(inputs["eigen_values"], dtype=np.float32)
    qf = np.asarray(inputs["queries_fmp"], dtype=np.float32)
    qe = np.asarray(inputs["queries_eig"], dtype=np.float32)
    bf = ml_dtypes.bfloat16

    q_all = np.concatenate([qf, qe], axis=0)  # [224, 4096]
    qt = np.ascontiguousarray(
        q_all.T.reshape(32, 128, M_PAD).transpose(1, 0, 2)
        .reshape(128, 32 * M_PAD).astype(bf))

    def conv_w(w):
        # [oc, ic, 3, 3] -> [ic, ky*3+kx, oc] flattened [ic, 9*oc], bf16
        oc, ic = w.shape[0], w.shape[1]
        return np.ascontiguousarray(
            np.asarray(w, np.float32).transpose(1, 2, 3, 0)
            .reshape(ic, 9 * oc).astype(bf))

    w1t = conv_w(inputs["w1"])
    w2t = conv_w(inputs["w2"])
    w3t = conv_w(inputs["w3"])
    w4t = conv_w(inputs["w4"])

    # lin_w packed to match the on-chip feature transpose:
    # fr[p, 8*b+j] = flat_b[f(p, j)] with f(p, j) = (p//2)*16 + (p%2)*8 + j
    lwt = np.asarray(inputs["lin_w"], np.float32).T  # [1024 f, 512 o]
    pp, jj = np.meshgrid(np.arange(128), np.arange(8), indexing="ij")
    feat = (pp // 2) * 16 + (pp % 2) * 8 + jj  # [128, 8]
    lw = np.ascontiguousarray(lwt[feat].reshape(128, 8 * 512).astype(bf))

    biasp = np.zeros((128, 5), np.float32)
    b1 = np.asarray(inputs["b1"], np.float32)
    biasp[:, 0] = b1[0:128]
    biasp[:, 1] = b1[128:256]
    biasp[:, 2] = np.asarray(inputs["b2"], np.float32)
    biasp[:, 3] = np.asarray(inputs["b3"], np.float32)
    biasp[0:64, 4] = np.asarray(inputs["b4"], np.float32)
    linb = np.zeros((2, 512), np.float32)
    linb[0] = np.asarray(inputs["lin_b"], np.float32)

    shared = {
        "qt": qt,
        "w1f": np.ascontiguousarray(w1t[0:96]),
        "w1e": np.ascontiguousarray(w1t[96:224]),
        "w2a": np.ascontiguousarray(w2t[0:128]),
        "w2b": np.ascontiguousarray(w2t[128:256]),
        "w3": w3t,
        "w4": w4t,
        "lw": lw,
        "biasp": biasp,
        "linb": linb.astype(bf),
        "onesc": np.ones((128, 2), np.float32),
        "onesr": np.ones((2, 4), bf),
        "ident": np.eye(128, dtype=bf),
    }

    in_maps = []
    for c in range(NCORES):
        sl = slice(c * BL, (c + 1) * BL)
        kc = keys[sl]
        ktc = np.ascontiguousarray(
            kc.transpose(0, 2, 1).reshape(BL, 32, 128, 128)
            .transpose(0, 2, 1, 3).reshape(BL, 128, D).astype(bf))
        scl = np.zeros((N, BL * 4), np.float32)
        for b in range(BL):
            scl[:, 4 * b + 0] = fmp[c * BL + b] * SCALE
            scl[:, 4 * b + 1] = eig[c * BL + b] * SCALE
            scl[:, 4 * b + 2] = fmp[c * BL + b]
            scl[:, 4 * b + 3] = eig[c * BL + b]
        m = {"keys": np.ascontiguousarray(kc.astype(bf)), "keysT": ktc,
             "scales": np.ascontiguousarray(scl)}
        m.update(shared)
        in_maps.append(m)
    return in_maps


def kernel(**inputs):
    global LAST_RESULT
    if "nc" not in _CACHE:
        _CACHE["nc"] = _build_program()
    nc = _CACHE["nc"]
    in_maps = _prep_inputs(inputs)
    res = bass_utils.run_bass_kernel_spmd(nc, in_maps, core_ids=list(range(NCORES)))
    LAST_RESULT = res
    out = np.concatenate([res.results[c]["out"] for c in range(NCORES)], axis=0)
    return out.astype(np.float32)


# revision 20
# speedup vs baseline: 1.1126x; 1.1126x over previous
"""Trainium2 Bass kernel for nn_ConditionalMomentEncoder.

Self-contained: takes full unsharded inputs, shards batch-parallel over 8
NeuronCores, runs one Bass/Tile program per core, gathers [32, 512] output.

Math notes (vs the jax reference):
- The ragged argsort/gather is eliminated: invalid slots have fmp == 0, so
  their key rows and logits are exactly 0. Attention over all N=128 slots
  with denominator correction  den = rowsum(exp(logits)) - (N - L)
  reproduces the reference's softmax over its L=96 gathered slots exactly.
- Softmax max-subtraction is skipped: logits are bounded, fp32 exp is safe.
- conv1 fold: conv1 is linear in its input channels and its input is
  O = A @ K (A = scaled attention weights [224, 128], K = raw keys
  [128, 4096] viewed as 128 images of 64x64). So per batch we compute
  W'[n, :] = sum_m A[m, n] * w1r[m, :]  (a [224]->[128] channel fold of the
  conv weights) and run conv1 directly on the keys images with W'. This
  skips the O matmuls entirely and roughly halves conv1's contraction.
- A is materialized via a PE transpose (matmul with identity rhs) of
  E^T[n, m] * s[n] (s = fmp/eig), scaled by 1/den[m] on evacuation.

Scheduling notes:
- PE executes in program order; phases are software-pipelined so that
  batch b+1's independent matmuls cover batch b's activation/evac stalls.
- A short warm-up matmul stream runs during the initial input DMA so the
  PE HAM clock gate is already at 2.4 GHz when real work arrives.
- The flatten transpose for the final linear is one small SBUF->SBUF DMA
  per batch (contiguous 16B runs per partition); the feature permutation
  is absorbed into the host-side lin_w packing.
"""

import sys

sys.path.insert(0, "/opt/trn_rl_repo")

import ml_dtypes
import numpy as np

import concourse.bacc as bacc
import concourse.mybir as mybir
import concourse.tile as tile
from concourse import bass_utils

F32 = mybir.dt.float32
F32R = mybir.dt.float32r
BF16 = mybir.dt.bfloat16
AF = mybir.ActivationFunctionType
ALU = mybir.AluOpType

B, N, D = 32, 128, 4096
L = 96
IMG = 64
NCORES = 8
BL = B // NCORES  # 4 batches per core
SCALE = float(D) ** -0.5
M_ALL = L + N  # 224 query rows total
M_PAD = 224

_CACHE = {}
LAST_RESULT = None


def _build_program():
    nc = bacc.Bacc("TRN2", target_bir_lowering=False, debug=False)

    keys_d = nc.dram_tensor("keys", [BL, N, D], BF16, kind="ExternalInput")
    keysT_d = nc.dram_tensor("keysT", [BL, N, D], BF16, kind="ExternalInput")
    scales_d = nc.dram_tensor("scales", [128, BL * 4], F32, kind="ExternalInput")
    qt_d = nc.dram_tensor("qt", [128, 32 * M_PAD], BF16, kind="ExternalInput")
    w1f_d = nc.dram_tensor("w1f", [96, 9 * 256], BF16, kind="ExternalInput")
    w1e_d = nc.dram_tensor("w1e", [128, 9 * 256], BF16, kind="ExternalInput")
    w2a_d = nc.dram_tensor("w2a", [128, 9 * 128], BF16, kind="ExternalInput")
    w2b_d = nc.dram_tensor("w2b", [128, 9 * 128], BF16, kind="ExternalInput")
    w3_d = nc.dram_tensor("w3", [128, 9 * 128], BF16, kind="ExternalInput")
    w4_d = nc.dram_tensor("w4", [128, 9 * 64], BF16, kind="ExternalInput")
    lw_d = nc.dram_tensor("lw", [128, 8 * 512], BF16, kind="ExternalInput")
    bias_d = nc.dram_tensor("biasp", [128, 5], F32, kind="ExternalInput")
    linb_d = nc.dram_tensor("linb", [2, 512], BF16, kind="ExternalInput")
    onesc_d = nc.dram_tensor("onesc", [128, 2], F32R, kind="ExternalInput")
    onesr_d = nc.dram_tensor("onesr", [2, 4], BF16, kind="ExternalInput")
    ident_d = nc.dram_tensor("ident", [128, 128], BF16, kind="ExternalInput")
    out_d = nc.dram_tensor("out", [BL, 512], F32, kind="ExternalOutput")
    warm_d = nc.dram_tensor("warm", [1, 1], F32, kind="ExternalOutput")

    with tile.TileContext(nc) as tc:
        with (
            tc.tile_pool(name="const", bufs=1) as cpool,
            tc.tile_pool(name="imgs", bufs=2) as ipool,
            tc.tile_pool(name="flat", bufs=1) as fpool,
            tc.tile_pool(name="keys", bufs=4) as kpool,
            tc.tile_pool(name="keysT", bufs=3) as ktpool,
            tc.tile_pool(name="small", bufs=4) as spool,
            tc.tile_pool(name="wprime", bufs=4) as wpool,
            tc.tile_pool(name="gt_ps", bufs=2, space="PSUM") as gt_psum,
            tc.tile_pool(name="tr_ps", bufs=1, space="PSUM") as tr_psum,
            tc.tile_pool(name="wp_ps", bufs=2, space="PSUM") as wp_psum,
            tc.tile_pool(name="cv_ps", bufs=3, space="PSUM") as cv_psum,
        ):
            # ---- startup DMAs: dispatch cost is ~0.6us per dma_start,
            # serialized per issuing engine, so keep the count low and split
            # across the two HWDGE queues (sync + scalar). The sync queue
            # carries the latency-critical front stream. ----
            ident = cpool.tile([128, 128], BF16)
            nc.sync.dma_start(ident[:], ident_d[:])
            keysT0 = ktpool.tile([128, D], BF16, tag="kt")
            qt_sb = cpool.tile([128, 32 * M_PAD], BF16)
            nc.sync.dma_start(keysT0[:, 0:2048], keysT_d[0][:, 0:2048])
            nc.sync.dma_start(qt_sb[:, 0:3584], qt_d[:, 0:3584])
            nc.sync.dma_start(keysT0[:, 2048:4096], keysT_d[0][:, 2048:4096])
            nc.sync.dma_start(qt_sb[:, 3584:7168], qt_d[:, 3584:7168])
            # sync stream in exact consumption order: HW queues serve
            # descriptors FIFO by dispatch order, so this fixes the HBM
            # transfer order too.
            ktt = {0: keysT0}
            kst = {}
            t = ktpool.tile([128, D], BF16, tag="kt")
            nc.sync.dma_start(t[:], keysT_d[1])
            ktt[1] = t
            w1f = cpool.tile([96, 9 * 256], BF16)
            nc.sync.dma_start(w1f[:], w1f_d[:])
            w1e = cpool.tile([128, 9 * 256], BF16)
            nc.sync.dma_start(w1e[:], w1e_d[:])
            t = ktpool.tile([128, D], BF16, tag="kt")
            nc.sync.dma_start(t[:], keysT_d[2])
            ktt[2] = t
            k = kpool.tile([128, D], BF16, tag="k")
            nc.sync.dma_start(k[:], keys_d[0])
            kst[0] = k
            t = ktpool.tile([128, D], BF16, tag="kt")
            nc.sync.dma_start(t[:], keysT_d[3])
            ktt[3] = t
            biasp = cpool.tile([128, 5], F32)
            nc.sync.dma_start(biasp[:], bias_d[:])
            k = kpool.tile([128, D], BF16, tag="k")
            nc.sync.dma_start(k[:], keys_d[1])
            kst[1] = k
            w2a = cpool.tile([128, 9 * 128], BF16)
            nc.sync.dma_start(w2a[:], w2a_d[:])
            w2b = cpool.tile([128, 9 * 128], BF16)
            nc.sync.dma_start(w2b[:], w2b_d[:])
            k = kpool.tile([128, D], BF16, tag="k")
            nc.sync.dma_start(k[:], keys_d[2])
            kst[2] = k
            w3 = cpool.tile([128, 9 * 128], BF16)
            nc.sync.dma_start(w3[:], w3_d[:])
            k = kpool.tile([128, D], BF16, tag="k")
            nc.sync.dma_start(k[:], keys_d[3])
            kst[3] = k
            w4 = cpool.tile([128, 9 * 64], BF16)
            nc.sync.dma_start(w4[:], w4_d[:])
            lw = cpool.tile([128, 8 * 512], BF16)
            nc.sync.dma_start(lw[:], lw_d[:])
            linb = cpool.tile([2, 512], BF16)
            nc.sync.dma_start(linb[:], linb_d[:])
            ones_row = cpool.tile([2, 4], BF16)
            nc.sync.dma_start(ones_row[:], onesr_d[:])

            # scalar queue: only the tiny early tensors
            scp = cpool.tile([128, BL * 4], F32)
            nc.scalar.dma_start(scp[:], scales_d[:])
            onesc = cpool.tile([128, 2], F32R)
            nc.scalar.dma_start(onesc[:], onesc_d[:])

            # flat features + transposed copy for the linear
            f_all = fpool.tile([64, BL * 16], BF16)
            fr = fpool.tile([128, 4 * 8], BF16)  # col = 8*b + j

            def rtap(ky, kx, S):
                # restricted tap for unpadded stride-2 conv, output side S
                oy0 = 1 if ky == 0 else 0
                ox0 = 1 if kx == 0 else 0
                return (oy0, S - oy0, ox0, S - ox0,
                        2 * oy0 + ky - 1, 2 * ox0 + kx - 1)
            TAPS1 = [(1, 1), (0, 0), (0, 1), (0, 2), (1, 0), (1, 2),
                     (2, 0), (2, 1), (2, 2)]

            # W' chunk boundaries (psum bank = 512 fp32)
            WCH = [(0, 512), (512, 1024), (1024, 1536), (1536, 2048),
                   (2048, 2304)]

            # ---- PE warm-up: ~38 matmuls on the identity while input DMAs
            # stream in, so HAM is at 2.4 GHz when real work starts ----
            warm_ps = cv_psum.tile([128, 128], F32, tag="cv")
            NWARM = 24
            for i in range(NWARM):
                nc.tensor.matmul(warm_ps[:], ident[:], ident[:],
                                 start=(i == 0), stop=(i == NWARM - 1))
            warm_sb = cpool.tile([1, 1], F32)
            nc.vector.tensor_copy(warm_sb[:], warm_ps[0:1, 0:1])
            nc.sync.dma_start(warm_d[:], warm_sb[:])

            # ======== phase A: attention + weight fold (pipelined) ========
            def logits(b):
                gt_ps = gt_psum.tile([128, M_PAD], F32, tag="gt")
                keysT_sb = ktt[b]
                for c in range(32):
                    nc.tensor.matmul(
                        gt_ps[:],
                        keysT_sb[:, c * 128:(c + 1) * 128],
                        qt_sb[:, c * M_PAD:(c + 1) * M_PAD],
                        start=(c == 0),
                        stop=(c == 31),
                    )
                return gt_ps

            def dentr(b, gt_ps):
                # den + PE transpose of scaled E^T -> A[m, n] (bf16)
                # E^T = exp(G^T * scale_col)
                et = spool.tile([128, M_ALL], F32R, tag="et")
                nc.scalar.activation(et[:, 0:96], gt_ps[:, 0:96], AF.Exp,
                                     bias=0.0, scale=scp[:, 4 * b:4 * b + 1])
                nc.scalar.activation(et[:, 96:224], gt_ps[:, 96:224], AF.Exp,
                                     bias=0.0, scale=scp[:, 4 * b + 1:4 * b + 2])

                # den[m] = sum_n E^T[n, m] (ones matmul)
                den_f_ps = tr_psum.tile([96, 2], F32, tag="tr")
                nc.tensor.matmul(den_f_ps[:], et[:, 0:96], onesc[:],
                                 start=True, stop=True)
                den_e_ps = tr_psum.tile([128, 2], F32, tag="tr")
                nc.tensor.matmul(den_e_ps[:], et[:, 96:224], onesc[:],
                                 start=True, stop=True)

                rden = spool.tile([128, 3], F32, tag="rden")
                nc.vector.tensor_scalar_add(rden[0:96, 0:1], den_f_ps[:, 0:1],
                                            -32.0)
                nc.vector.reciprocal(rden[0:96, 1:2], rden[0:96, 0:1])
                nc.vector.reciprocal(rden[0:128, 2:3], den_e_ps[:, 0:1])

                # scaled E^T (bf16): et2[n, m] = E^T[n, m] * s[n]
                et2 = spool.tile([128, M_ALL], BF16, tag="et2")
                nc.vector.tensor_scalar_mul(et2[:, 0:96], et[:, 0:96],
                                            scp[:, 4 * b + 2:4 * b + 3])
                nc.vector.tensor_scalar_mul(et2[:, 96:224], et[:, 96:224],
                                            scp[:, 4 * b + 3:4 * b + 4])

                # A[m, n] = et2^T[m, n] / den[m] via PE transpose
                tr_f_ps = tr_psum.tile([96, 128], F32, tag="tr")
                nc.tensor.matmul(tr_f_ps[:], et2[:, 0:96], ident[:],
                                 start=True, stop=True)
                tr_e_ps = tr_psum.tile([128, 128], F32, tag="tr")
                nc.tensor.matmul(tr_e_ps[:], et2[:, 96:224], ident[:],
                                 start=True, stop=True)
                at_f = spool.tile([96, 128], BF16, tag="atf")
                nc.scalar.activation(at_f[:], tr_f_ps[:], AF.Copy,
                                     bias=0.0, scale=rden[0:96, 1:2])
                at_e = spool.tile([128, 128], BF16, tag="ate")
                nc.scalar.activation(at_e[:], tr_e_ps[:], AF.Copy,
                                     bias=0.0, scale=rden[0:128, 2:3])
                return at_f, at_e

            def wfold(ats):
                # W'[n, 9*256] = A_f^T @ w1f + A_e^T @ w1e
                at_f, at_e = ats
                wp = wpool.tile([128, 9 * 256], BF16, tag="wp")
                for c0, c1 in WCH:
                    wp_ps = wp_psum.tile([128, 512], F32, tag="wp")
                    nc.tensor.matmul(wp_ps[:, 0:c1 - c0], at_f[:],
                                     w1f[:, c0:c1], start=True, stop=False)
                    nc.tensor.matmul(wp_ps[:, 0:c1 - c0], at_e[:],
                                     w1e[:, c0:c1], start=False, stop=True)
                    nc.vector.tensor_copy(wp[:, c0:c1], wp_ps[:, 0:c1 - c0])
                return wp

            gts = {}
            ats = {}
            wpt = {}

            # ======== phase C: conv stack (cross-batch interleaved) ========
            c1t = {}

            def conv1_quarter(b, og, fc):
                # conv1': 128 keys-ch -> 128 of 256 ch, 64x64 -> 32x32
                if b not in c1t:
                    ca = ipool.tile([128, 1024], BF16, tag="c1a")
                    cb = ipool.tile([128, 1024], BF16, tag="c1b")
                    c1t[b] = (ca, cb)
                wp = wpt[b]
                keys3 = kst[b][:].rearrange("p (h w) -> p h w", h=64, w=64)
                ps = cv_psum.tile([128, 512], F32, tag="cv")
                ps3 = ps[:].rearrange("p (a b) -> p a b", a=16, b=32)
                for ti, (ky, kx) in enumerate(TAPS1):
                    oy0 = 1 if (ky == 0 and fc == 0) else 16 * fc
                    ny = 16 * (fc + 1) - oy0
                    ox0 = 1 if kx == 0 else 0
                    nx = 32 - ox0
                    iy0 = 2 * oy0 + ky - 1
                    ix0 = 2 * ox0 + kx - 1
                    nc.tensor.matmul(
                        ps3[:, oy0 - 16 * fc:oy0 - 16 * fc + ny, ox0:32],
                        wp[:, (3 * ky + kx) * 256 + og * 128:
                           (3 * ky + kx) * 256 + og * 128 + 128],
                        keys3[:, iy0:iy0 + 2 * ny - 1:2,
                              ix0:ix0 + 2 * nx - 1:2],
                        start=(ti == 0),
                        stop=(ti == 8),
                    )
                c1dst = c1t[b][og]
                nc.scalar.activation(
                    c1dst[:, 512 * fc:512 * (fc + 1)], ps[:],
                    AF.Relu, bias=biasp[:, og:og + 1], scale=1.0,
                )

            def conv2(b):
                ca, cb = c1t[b]
                c1a3 = ca[:].rearrange("p (h w) -> p h w", h=32, w=32)
                c1b3 = cb[:].rearrange("p (h w) -> p h w", h=32, w=32)
                ps = cv_psum.tile([128, 256], F32, tag="cv")
                ps3 = ps[:].rearrange("p (a b) -> p a b", a=16, b=16)
                for ti, (ky, kx) in enumerate(TAPS1):
                    rr = rtap(ky, kx, 16)
                    for ic in range(2):
                        srci = c1a3 if ic == 0 else c1b3
                        wt = w2a if ic == 0 else w2b
                        nc.tensor.matmul(
                            ps3[:, rr[0]:rr[0] + rr[1], rr[2]:rr[2] + rr[3]],
                            wt[:, (3 * ky + kx) * 128:(3 * ky + kx) * 128 + 128],
                            srci[:, rr[4]:rr[4] + 2 * rr[1] - 1:2,
                                 rr[5]:rr[5] + 2 * rr[3] - 1:2],
                            start=(ti == 0 and ic == 0),
                            stop=(ti == 8 and ic == 1),
                        )
                c2o = ipool.tile([128, 256], BF16, tag="c2o")
                nc.scalar.activation(c2o[:], ps[:], AF.Relu,
                                     bias=biasp[:, 2:3], scale=1.0)
                return c2o

            def conv3(b, c2o):
                c2o3 = c2o[:].rearrange("p (h w) -> p h w", h=16, w=16)
                ps = cv_psum.tile([128, 64], F32, tag="cv")
                ps3 = ps[:].rearrange("p (a b) -> p a b", a=8, b=8)
                for ti, (ky, kx) in enumerate(TAPS1):
                    rr = rtap(ky, kx, 8)
                    nc.tensor.matmul(
                        ps3[:, rr[0]:rr[0] + rr[1], rr[2]:rr[2] + rr[3]],
                        w3[:, (3 * ky + kx) * 128:(3 * ky + kx) * 128 + 128],
                        c2o3[:, rr[4]:rr[4] + 2 * rr[1] - 1:2,
                             rr[5]:rr[5] + 2 * rr[3] - 1:2],
                        start=(ti == 0), stop=(ti == 8),
                    )
                c3o = ipool.tile([128, 64], BF16, tag="c3o")
                nc.scalar.activation(c3o[:], ps[:], AF.Relu,
                                     bias=biasp[:, 3:4], scale=1.0)
                return c3o

            def conv4(b, c3o):
                c3o3 = c3o[:].rearrange("p (h w) -> p h w", h=8, w=8)
                ps = cv_psum.tile([64, 16], F32, tag="cv")
                ps3 = ps[:].rearrange("p (a b) -> p a b", a=4, b=4)
                for ti, (ky, kx) in enumerate(TAPS1):
                    rr = rtap(ky, kx, 4)
                    nc.tensor.matmul(
                        ps3[:, rr[0]:rr[0] + rr[1], rr[2]:rr[2] + rr[3]],
                        w4[:, (3 * ky + kx) * 64:(3 * ky + kx) * 64 + 64],
                        c3o3[:, rr[4]:rr[4] + 2 * rr[1] - 1:2,
                             rr[5]:rr[5] + 2 * rr[3] - 1:2],
                        start=(ti == 0), stop=(ti == 8),
                    )
                nc.scalar.activation(f_all[:, b * 16:(b + 1) * 16], ps3[:],
                                     AF.Relu, bias=biasp[0:64, 4:5], scale=1.0)
                # feature transpose to 128 partitions, one SBUF->SBUF DMA:
                # (ch, s) -> (p = 2*ch + s//8, col = 8*b + s%8); lw host
                # packing matches this ordering.
                nc.sync.dma_start(fr[:, b * 8:(b + 1) * 8],
                                  f_all[:, b * 16:(b + 1) * 16])

            # ---- pipelined emission (PE executes in program order):
            # the logits chain is DMA-paced at the front, so batch-0's conv
            # quarters and later batches' den/fold work are interleaved to
            # keep the PE dense; conv2..4 of batch b sit between batch b+1's
            # conv1 quarters so relu-evac latencies are covered.
            gts[0] = logits(0)
            gts[1] = logits(1)
            ats[0] = dentr(0, gts[0])
            wpt[0] = wfold(ats[0])
            gts[2] = logits(2)
            ats[1] = dentr(1, gts[1])
            wpt[1] = wfold(ats[1])
            conv1_quarter(0, 0, 0)
            conv1_quarter(0, 0, 1)
            gts[3] = logits(3)
            conv1_quarter(0, 1, 0)
            ats[2] = dentr(2, gts[2])
            wpt[2] = wfold(ats[2])
            conv1_quarter(0, 1, 1)
            ats[3] = dentr(3, gts[3])
            wpt[3] = wfold(ats[3])
            for b in range(BL):
                if b + 1 < BL:
                    conv1_quarter(b + 1, 0, 0)
                    conv1_quarter(b + 1, 0, 1)
                    c2o = conv2(b)
                    conv1_quarter(b + 1, 1, 0)
                    c3o = conv3(b, c2o)
                    conv1_quarter(b + 1, 1, 1)
                    conv4(b, c3o)
                else:
                    c2o = conv2(b)
                    c3o = conv3(b, c2o)
                    conv4(b, c3o)

            # ---- linear: out[b, o] = sum_f flat[b, f] lin_w[o, f] + lin_b ----
            lin_ps = cv_psum.tile([4, 512], F32, tag="cv")
            for j in range(8):
                nc.tensor.matmul(
                    lin_ps[:],
                    fr[:, j::8],
                    lw[:, 512 * j:512 * j + 512],
                    start=(j == 0), stop=False,
                )
            nc.tensor.matmul(lin_ps[:], ones_row[:], linb[:],
                             start=False, stop=True)
            out_sb = cpool.tile([4, 512], F32)
            nc.vector.tensor_copy(out_sb[:], lin_ps[:])
            nc.sync.dma_start(out_d[:], out_sb[:])

    nc.finalize()
    return nc


def _prep_inputs(inputs):
    keys = np.ascontiguousarray(inputs["keys"], dtype=np.float32)
    fmp = np.asarray(inputs["first_moment_projections"], dtype=np.float32)
    eig = np.asarray(inputs["eigen_values"], dtype=np.float32)
    qf = np.asarray(inputs["queries_fmp"], dtype=np.float32)
    qe = np.asarray(inputs["queries_eig"], dtype=np.float32)
    bf = ml_dtypes.bfloat16

    q_all = np.concatenate([qf, qe], axis=0)  # [224, 4096]
    qt = np.ascontiguousarray(
        q_all.T.reshape(32, 128, M_PAD).transpose(1, 0, 2)
        .reshape(128, 32 * M_PAD).astype(bf))

    def conv_w(w):
        # [oc, ic, 3, 3] -> [ic, ky*3+kx, oc] flattened [ic, 9*oc], bf16
        oc, ic = w.shape[0], w.shape[1]
        return np.ascontiguousarray(
            np.asarray(w, np.float32).transpose(1, 2, 3, 0)
            .reshape(ic, 9 * oc).astype(bf))

    w1t = conv_w(inputs["w1"])
    w2t = conv_w(inputs["w2"])
    w3t = conv_w(inputs["w3"])
    w4t = conv_w(inputs["w4"])

    # lin_w packed to match the on-chip feature transpose:
    # fr[p, 8*b+j] = flat_b[f(p, j)] with f(p, j) = (p//2)*16 + (p%2)*8 + j
    lwt = np.asarray(inputs["lin_w"], np.float32).T  # [1024 f, 512 o]
    pp, jj = np.meshgrid(np.arange(128), np.arange(8), indexing="ij")
    feat = (pp // 2) * 16 + (pp % 2) * 8 + jj  # [128, 8]
    lw = np.ascontiguousarray(lwt[feat].reshape(128, 8 * 512).astype(bf))

    biasp = np.zeros((128, 5), np.float32)
    b1 = np.asarray(inputs["b1"], np.float32)
    biasp[:, 0] = b1[0:128]
    biasp[:, 1] = b1[128:256]
    biasp[:, 2] = np.asarray(inputs["b2"], np.float32)
    biasp[:, 3] = np.asarray(inputs["b3"], np.float32)
    biasp[0:64, 4] = np.asarray(inputs["b4"], np.float32)
    linb = np.zeros((2, 512), np.float32)
    linb[0] = np.asarray(inputs["lin_b"], np.float32)

    shared = {
        "qt": qt,
        "w1f": np.ascontiguousarray(w1t[0:96]),
        "w1e": np.ascontiguousarray(w1t[96:224]),
        "w2a": np.ascontiguousarray(w2t[0:128]),
        "w2b": np.ascontiguousarray(w2t[128:256]),
        "w3": w3t,
        "w4": w4t,
        "lw": lw,
        "biasp": biasp,
        "linb": linb.astype(bf),
        "onesc": np.ones((128, 2), np.float32),
        "onesr": np.ones((2, 4), bf),
        "ident": np.eye(128, dtype=bf),
    }

    in_maps = []
    for c in range(NCORES):
        sl = slice(c * BL, (c + 1) * BL)
        kc = keys[sl]
        ktc = np.ascontiguousarray(
            kc.transpose(0, 2, 1).reshape(BL, 32, 128, 128)
            .transpose(0, 2, 1, 3).reshape(BL, 128, D).astype(bf))
        scl = np.zeros((N, BL * 4), np.float32)
        for b in range(BL):
            scl[:, 4 * b + 0] = fmp[c * BL + b] * SCALE
            scl[:, 4 * b + 1] = eig[c * BL + b] * SCALE
            scl[:, 4 * b + 2] = fmp[c * BL + b]
            scl[:, 4 * b + 3] = eig[c * BL + b]
        m = {"keys": np.ascontiguousarray(kc.astype(bf)), "keysT": ktc,
             "scales": np.ascontiguousarray(scl)}
        m.update(shared)
        in_maps.append(m)
    return in_maps


def kernel(**inputs):
    global LAST_RESULT
    if "nc" not in _CACHE:
        _CACHE["nc"] = _build_program()
    nc = _CACHE["nc"]
    in_maps = _prep_inputs(inputs)
    res = bass_utils.run_bass_kernel_spmd(nc, in_maps, core_ids=list(range(NCORES)))
    LAST_RESULT = res
    out = np.concatenate([res.results[c]["out"] for c in range(NCORES)], axis=0)
    return out.astype(np.float32)
